# revision 26
# baseline (speedup 1.0000x reference)
"""MoE ExpertCombiner (scatter-add) Trainium2 Bass kernel.

  out[b, s, :] = sum over (e, c) with token_indices[e,c] == b*S+s of
                 weights[e, c] * expert_outputs[e, c, :]

Strategy (8 NeuronCores, SPMD):
  Host: flatten the (e, c) rows, stable-sort by destination token, and
  shard the TOKEN space contiguously across the 8 cores (each core owns
  4096 destination tokens and receives exactly the sorted rows that land
  in its range -> no cross-core reduction at all; outputs concatenate).

  Device: the scatter-add becomes block-diagonal one-hot matmuls.  For
  each 128-token output window, PSUM accumulates
      onehot[rows_chunk, 128].T @ x[rows_chunk, D]
  over the few 128-row chunks of the sorted stream that overlap the
  window.  The routing weight is folded into the one-hot (entries are
  w_r instead of 1), built on VectorE in ONE tensor_scalar op per chunk:
      oh[p, j] = (iota[j] == idx[p]) * w[p]
  Rows are shipped as fp16 and the onehot is fp16, so each pair needs a
  single full-rate 16-bit matmul per 512-wide PSUM half (vs hi+lo f32r
  pairs for full fp32).  Completed windows are converted fp32->fp16
  PSUM->SBUF (VectorE takes bank 0, ScalarE bank 1 - parallel PSUM
  access), batched 4 windows per ~1MB store.  The host upconverts to
  fp32.  End-to-end relative error ~1e-4 (fp16 rounding), well inside
  the 2e-2 gate.

Per-core traffic is ~18MB in + 8.4MB out vs the ~48MB of the all-fp32
variant; PE work is ~4x less than the hi/lo fp32 scheme.
"""

import math

import numpy as np

import concourse.bacc as bacc
import concourse.mybir as mybir
import concourse.tile as tile
from concourse import bass_utils

P = 128
F32 = mybir.dt.float32
F16 = mybir.dt.float16

N_CORES = 8
W_TOK = 128


def _make_plan(idx_flat, n_tokens, n_cores, w_tok=128, group_chunks=4):
    """Sort/shard/window planning. Returns plan dict (shared across cores)."""
    order = np.argsort(idx_flat, kind="stable")
    idx_s = idx_flat[order]
    tok_per_core = n_tokens // n_cores
    n_win = tok_per_core // w_tok
    bounds = np.searchsorted(idx_s, np.arange(n_cores + 1) * tok_per_core)
    counts = np.diff(bounds)
    R = int(counts.max())
    nchunk = math.ceil(R / P)
    nchunk = math.ceil(nchunk / group_chunks) * group_chunks
    npad = nchunk * P

    c_lo = np.full(n_win, 1 << 30, np.int64)
    c_hi = np.full(n_win, -1, np.int64)
    for m in range(n_cores):
        il = idx_s[bounds[m]:bounds[m + 1]] - m * tok_per_core
        ws = np.searchsorted(il, np.arange(n_win + 1) * w_tok)
        s_, e_ = ws[:-1], ws[1:]
        ne = e_ > s_
        c_lo[ne] = np.minimum(c_lo[ne], s_[ne] // P)
        c_hi[ne] = np.maximum(c_hi[ne], (e_[ne] - 1) // P)
    c_lo = np.clip(c_lo, 0, nchunk - 1)
    c_hi = np.clip(c_hi, 0, nchunk - 1)
    c_hi = np.maximum(c_hi, c_lo)

    pairs = []
    win_pair_slices = []
    for w in range(n_win):
        s = len(pairs)
        for c in range(int(c_lo[w]), int(c_hi[w]) + 1):
            pairs.append((w, c))
        win_pair_slices.append((s, len(pairs)))

    chunk_wfirst = {}
    chunk_span = {}
    for w, c in pairs:
        if c not in chunk_wfirst:
            chunk_wfirst[c] = w
        chunk_span[c] = w - chunk_wfirst[c] + 1
    w_span = max(chunk_span.values()) if chunk_span else 1
    live = int(c_hi.max()) + 1  # chunks >= live are never matmul'd

    return dict(
        live=live,
        order=order, idx_s=idx_s, bounds=bounds, n_win=n_win, w_tok=w_tok,
        tok_per_core=tok_per_core, nchunk=nchunk, npad=npad, pairs=pairs,
        win_pair_slices=win_pair_slices, n_cores=n_cores,
        group_chunks=group_chunks, chunk_wfirst=chunk_wfirst,
        chunk_span=chunk_span, w_span=w_span,
    )


def _pack_core_inputs(plan, m, x_flat, w_flat, D):
    """Build in_map arrays for core m.

    rows: fp16, [P, ngrp*gch*D] with chunk k of group g at columns
          (g*gch + k)*D, partition p = row (within-chunk).
    meta layout: [128, nchunk * 2] f32
      cols [0, nchunk)          : per-chunk weight column
      cols [nchunk, 2*nchunk)   : per-chunk window-relative index column
    """
    order, idx_s, bounds = plan["order"], plan["idx_s"], plan["bounds"]
    npad, nchunk = plan["npad"], plan["nchunk"]
    w_tok, tok_per_core = plan["w_tok"], plan["tok_per_core"]
    gch = plan["group_chunks"]
    sel = order[bounds[m]:bounds[m + 1]]
    Rm = len(sel)
    rows = np.zeros((npad, D), np.float16)
    rows[:Rm] = x_flat[sel].astype(np.float16)
    ngrp = npad // (P * gch)
    # group-contiguous partition-major layout: group g occupies one
    # contiguous 1MB DRAM block (sequential HBM reads per descriptor)
    rows = np.ascontiguousarray(
        rows.reshape(ngrp, gch, P, D).transpose(0, 2, 1, 3)
    ).reshape(ngrp * P, gch * D)
    wv = np.zeros(npad, np.float32)
    wv[:Rm] = w_flat[sel]
    # fp16 sentinel: any value outside [0, w_span*w_tok) kills the compare
    il = np.full(npad, -1024.0, np.float32)
    il[:Rm] = (idx_s[bounds[m]:bounds[m + 1]] - m * tok_per_core).astype(np.float32)

    meta = np.zeros((P, nchunk * 2), np.float32)
    meta[:, :nchunk] = wv.reshape(nchunk, P).T
    ilm = il.reshape(nchunk, P).T.copy()
    for c, wf in plan["chunk_wfirst"].items():
        ilm[:, c] -= wf * w_tok
    meta[:, nchunk:] = ilm

    wide = plan["w_span"] * w_tok
    iota = np.broadcast_to(np.arange(wide, dtype=np.float16), (P, wide)).copy()
    return {"rows": rows, "meta": meta, "iota": iota}


def _store_sizes(n_win, store_w=4):
    if n_win >= 2 * store_w and n_win % store_w == 0:
        sizes = [store_w] * (n_win // store_w - 1) + [2, 1, 1]
    else:
        sizes = [store_w] * (n_win // store_w)
    assert sum(sizes) == n_win
    return sizes


def _build_program(plan, D, n_cores, group_bufs=6, stage_bufs=3,
                   psum_bufs=4, onehot_bufs=10, store_w=4):
    n_win, w_tok = plan["n_win"], plan["w_tok"]
    nchunk, npad = plan["nchunk"], plan["npad"]
    pairs, win_pair_slices = plan["pairs"], plan["win_pair_slices"]
    gch = plan["group_chunks"]
    chunk_wfirst = plan["chunk_wfirst"]
    chunk_span = plan["chunk_span"]
    w_span = plan["w_span"]
    live = plan["live"]
    half = min(D, 512)
    n_half = D // half
    eq = mybir.AluOpType.is_equal

    ngrp = nchunk // gch
    nc = bacc.Bacc("TRN2", target_bir_lowering=False, debug=False,
                   enable_asserts=False, num_devices=n_cores)
    rows_d = nc.dram_tensor("rows", [ngrp * P, gch * D], F16,
                            kind="ExternalInput").ap()
    meta_d = nc.dram_tensor("meta", [P, nchunk * 2], F32,
                            kind="ExternalInput").ap()
    iota_d = nc.dram_tensor("iota", [P, w_span * w_tok], F16,
                            kind="ExternalInput").ap()
    # flat output: store block at window w0 of size sz lands at
    # [w0*P*D, (w0+sz)*P*D) in (partition, window, d) order; host decodes
    out_d = nc.dram_tensor("out", [1, n_win * P * D], F16,
                           kind="ExternalOutput").ap()

    with tile.TileContext(nc) as tc:
        with (
            tc.tile_pool(name="grp", bufs=group_bufs) as gpool,
            tc.tile_pool(name="misc", bufs=1) as mpool,
            tc.tile_pool(name="stage", bufs=stage_bufs) as spool,
            tc.tile_pool(name="oh", bufs=onehot_bufs) as opool,
            tc.tile_pool(name="ps", bufs=psum_bufs, space="PSUM") as ppool,
        ):
            iota_t = mpool.tile([P, w_span * w_tok], F16)
            nc.gpsimd.dma_start(out=iota_t[:], in_=iota_d[:])
            meta_t = mpool.tile([P, nchunk * 2], F32)
            nc.gpsimd.dma_start(out=meta_t[:], in_=meta_d[:])

            group_tiles = {}
            oh_tiles = {}

            def get_group(g):
                t = group_tiles.get(g)
                if t is None:
                    t = gpool.tile([P, gch * D], F16, tag="grp")
                    rsl = slice(g * P, (g + 1) * P)
                    # chunks >= live never reach a matmul; skip their DMA.
                    # (their buffer slot holds finite leftovers, harmless)
                    nliv = min(gch, live - g * gch)
                    # alternate the two HWDGE rings (sync=SP, scalar=ACT)
                    # so both descriptor rings feed the SDMA engines
                    eng = nc.sync if g % 2 == 0 else nc.scalar
                    if g < 2:
                        # per-chunk loads so chunk 0 lands ASAP
                        for j in range(nliv):
                            eng.dma_start(
                                out=t[:, j * D:(j + 1) * D],
                                in_=rows_d[rsl, j * D:(j + 1) * D],
                            )
                    elif nliv > 0:
                        eng.dma_start(
                            out=t[:, :nliv * D],
                            in_=rows_d[rsl, :nliv * D],
                        )
                    group_tiles[g] = t
                return t

            def get_oh(c):
                """Weighted one-hot for chunk c over its window span."""
                t = oh_tiles.get(c)
                if t is None:
                    t = opool.tile([P, w_span * w_tok], F16, tag="oh")
                    ncols = chunk_span.get(c, 1) * w_tok
                    nc.vector.tensor_scalar(
                        t[:, :ncols], iota_t[:, :ncols],
                        meta_t[:, nchunk + c:nchunk + c + 1],
                        meta_t[:, c:c + 1],
                        op0=eq, op1=mybir.AluOpType.mult,
                    )
                    oh_tiles[c] = t
                return t

            # store groups: big batched stores, tapered at the end so the
            # final window's store is small (shorter drain tail)
            sizes = _store_sizes(n_win, store_w)

            w = 0
            for sg, sz in enumerate(sizes):
                st = spool.tile([P, store_w * D], F16, tag="st")
                for sw in range(sz):
                    ps = ppool.tile([P, D], F32)
                    s, e = win_pair_slices[w]
                    for j in range(s, e):
                        _, c = pairs[j]
                        first, last = (j == s), (j == e - 1)
                        oh = get_oh(c)
                        g, k = divmod(c, gch)
                        gt = get_group(g)
                        off = (w - chunk_wfirst[c]) * w_tok
                        ohs = oh[:, off:off + w_tok]
                        for h in range(n_half):
                            hs = slice(h * half, (h + 1) * half)
                            nc.tensor.matmul(ps[:, hs],
                                             ohs,
                                             gt[:, k * D + h * half:
                                                k * D + (h + 1) * half],
                                             start=first, stop=last)
                    # fp32 PSUM -> fp16 SBUF; VectorE bank 0, ScalarE bank 1
                    hd = D // 2
                    nc.vector.tensor_copy(st[:, sw * D:sw * D + hd],
                                          ps[:, :hd])
                    nc.scalar.activation(st[:, sw * D + hd:(sw + 1) * D],
                                         ps[:, hd:],
                                         mybir.ActivationFunctionType.Copy)
                    w += 1
                seng = nc.scalar if sg % 2 == 0 else nc.sync
                seng.dma_start(
                    out=out_d[:, (w - sz) * P * D:w * P * D],
                    in_=st[:, :sz * D])

    nc.compile()
    return nc


def kernel(expert_outputs, weights, token_indices, batch_size, seq_len):
    expert_outputs = np.ascontiguousarray(expert_outputs, dtype=np.float32)
    weights = np.ascontiguousarray(weights, dtype=np.float32)
    B, S = int(batch_size), int(seq_len)
    E, C, D = expert_outputs.shape
    n_tokens = B * S

    x_flat = expert_outputs.reshape(-1, D)
    w_flat = weights.reshape(-1)
    idx_flat = np.asarray(token_indices).reshape(-1).astype(np.int64)

    plan = _make_plan(idx_flat, n_tokens, N_CORES)
    in_maps = [_pack_core_inputs(plan, m, x_flat, w_flat, D)
               for m in range(N_CORES)]
    nc = _build_program(plan, D, N_CORES)

    res = bass_utils.run_bass_kernel_spmd(
        nc, in_maps, core_ids=list(range(N_CORES)), trace=False,
    )
    tok_per_core = plan["tok_per_core"]
    n_win = plan["n_win"]
    sizes = _store_sizes(n_win)
    out = np.empty((n_tokens, D), np.float32)
    for m in range(N_CORES):
        o = np.asarray(res.results[m]["out"]).reshape(-1)  # flat fp16
        w0 = 0
        for sz in sizes:
            seg = (o[w0 * P * D:(w0 + sz) * P * D]
                   .reshape(P, sz, D).transpose(1, 0, 2)
                   .reshape(sz * P, D))
            t0 = m * tok_per_core + w0 * P
            out[t0:t0 + sz * P] = seg
            w0 += sz
    return out.reshape(B, S, D)


# revision 35
# speedup vs baseline: 1.2442x; 1.2442x over previous
"""MoE ExpertCombiner (scatter-add) Trainium2 Bass kernel.

  out[b, s, :] = sum over (e, c) with token_indices[e,c] == b*S+s of
                 weights[e, c] * expert_outputs[e, c, :]

Strategy (8 NeuronCores, SPMD):
  Host: flatten the (e, c) rows, stable-sort by destination token, and
  shard the TOKEN space contiguously across the 8 cores (each core owns
  4096 destination tokens and receives exactly the sorted rows that land
  in its range -> no cross-core reduction at all; outputs concatenate).

  Device: the scatter-add becomes block-diagonal one-hot matmuls.  For
  each 128-token output window, PSUM accumulates
      onehot[rows_chunk, 128].T @ x[rows_chunk, D]
  over the few 128-row chunks of the sorted stream that overlap the
  window.  The routing weight is folded into the one-hot (entries are
  w_r instead of 1), built on VectorE in ONE tensor_scalar op per chunk:
      oh[p, j] = (iota[j] == idx[p]) * w[p]
  Rows are shipped as fp16 and the onehot is fp16, so each pair needs a
  single full-rate 16-bit matmul per 512-wide PSUM half (vs hi+lo f32r
  pairs for full fp32).  Completed windows are converted fp32->fp16
  PSUM->SBUF (VectorE takes bank 0, ScalarE bank 1 - parallel PSUM
  access), batched 4 windows per ~1MB store.  The host upconverts to
  fp32.  End-to-end relative error ~1e-4 (fp16 rounding), well inside
  the 2e-2 gate.

Per-core traffic is ~18MB in + 8.4MB out vs the ~48MB of the all-fp32
variant; PE work is ~4x less than the hi/lo fp32 scheme.
"""

import math

import numpy as np

import concourse.bacc as bacc
import concourse.mybir as mybir
import concourse.tile as tile
from concourse import bass_utils

P = 128
F32 = mybir.dt.float32
F16 = mybir.dt.float16

N_CORES = 8
W_TOK = 128


def _make_plan(idx_flat, n_tokens, n_cores, w_tok=128, group_chunks=4):
    """Sort/shard/window planning. Returns plan dict (shared across cores)."""
    order = np.argsort(idx_flat, kind="stable")
    idx_s = idx_flat[order]
    tok_per_core = n_tokens // n_cores
    n_win = tok_per_core // w_tok
    bounds = np.searchsorted(idx_s, np.arange(n_cores + 1) * tok_per_core)
    counts = np.diff(bounds)
    R = int(counts.max())
    nchunk = math.ceil(R / P)
    nchunk = math.ceil(nchunk / group_chunks) * group_chunks
    npad = nchunk * P

    c_lo = np.full(n_win, 1 << 30, np.int64)
    c_hi = np.full(n_win, -1, np.int64)
    for m in range(n_cores):
        il = idx_s[bounds[m]:bounds[m + 1]] - m * tok_per_core
        ws = np.searchsorted(il, np.arange(n_win + 1) * w_tok)
        s_, e_ = ws[:-1], ws[1:]
        ne = e_ > s_
        c_lo[ne] = np.minimum(c_lo[ne], s_[ne] // P)
        c_hi[ne] = np.maximum(c_hi[ne], (e_[ne] - 1) // P)
    c_lo = np.clip(c_lo, 0, nchunk - 1)
    c_hi = np.clip(c_hi, 0, nchunk - 1)
    c_hi = np.maximum(c_hi, c_lo)

    pairs = []
    win_pair_slices = []
    for w in range(n_win):
        s = len(pairs)
        for c in range(int(c_lo[w]), int(c_hi[w]) + 1):
            pairs.append((w, c))
        win_pair_slices.append((s, len(pairs)))

    chunk_wfirst = {}
    chunk_span = {}
    for w, c in pairs:
        if c not in chunk_wfirst:
            chunk_wfirst[c] = w
        chunk_span[c] = w - chunk_wfirst[c] + 1
    w_span = max(chunk_span.values()) if chunk_span else 1
    live = int(c_hi.max()) + 1  # chunks >= live are never matmul'd

    return dict(
        live=live,
        order=order, idx_s=idx_s, bounds=bounds, n_win=n_win, w_tok=w_tok,
        tok_per_core=tok_per_core, nchunk=nchunk, npad=npad, pairs=pairs,
        win_pair_slices=win_pair_slices, n_cores=n_cores,
        group_chunks=group_chunks, chunk_wfirst=chunk_wfirst,
        chunk_span=chunk_span, w_span=w_span,
    )


def _pack_core_inputs(plan, m, x_flat, w_flat, D):
    """Build in_map arrays for core m.

    rows: fp16, [P, ngrp*gch*D] with chunk k of group g at columns
          (g*gch + k)*D, partition p = row (within-chunk).
    meta layout: [128, nchunk * 2] f32
      cols [0, nchunk)          : per-chunk weight column
      cols [nchunk, 2*nchunk)   : per-chunk window-relative index column
    """
    order, idx_s, bounds = plan["order"], plan["idx_s"], plan["bounds"]
    npad, nchunk = plan["npad"], plan["nchunk"]
    w_tok, tok_per_core = plan["w_tok"], plan["tok_per_core"]
    gch = plan["group_chunks"]
    sel = order[bounds[m]:bounds[m + 1]]
    Rm = len(sel)
    rows = np.zeros((npad, D), np.float16)
    rows[:Rm] = x_flat[sel].astype(np.float16)
    ngrp = npad // (P * gch)
    rows = np.ascontiguousarray(
        rows.reshape(ngrp, gch, P, D).transpose(2, 0, 1, 3)
    ).reshape(P, ngrp * gch * D)
    wv = np.zeros(npad, np.float32)
    wv[:Rm] = w_flat[sel]
    # fp16 sentinel: any value outside [0, w_span*w_tok) kills the compare
    il = np.full(npad, -1024.0, np.float32)
    il[:Rm] = (idx_s[bounds[m]:bounds[m + 1]] - m * tok_per_core).astype(np.float32)

    meta = np.zeros((P, nchunk * 2), np.float32)
    meta[:, :nchunk] = wv.reshape(nchunk, P).T
    ilm = il.reshape(nchunk, P).T.copy()
    for c, wf in plan["chunk_wfirst"].items():
        ilm[:, c] -= wf * w_tok
    meta[:, nchunk:] = ilm

    wide = plan["w_span"] * w_tok
    iota = np.broadcast_to(np.arange(wide, dtype=np.float16), (P, wide)).copy()
    return {"rows": rows, "meta": meta, "iota": iota}


def _store_sizes(n_win, store_w=4):
    if n_win >= 2 * store_w and n_win % store_w == 0:
        sizes = [store_w] * (n_win // store_w - 1) + [2, 1, 1]
    else:
        sizes = [store_w] * (n_win // store_w)
    assert sum(sizes) == n_win
    return sizes


def _build_program(plan, D, n_cores, group_bufs=6, stage_bufs=3,
                   psum_bufs=4, onehot_bufs=10, store_w=4):
    n_win, w_tok = plan["n_win"], plan["w_tok"]
    nchunk, npad = plan["nchunk"], plan["npad"]
    pairs, win_pair_slices = plan["pairs"], plan["win_pair_slices"]
    gch = plan["group_chunks"]
    chunk_wfirst = plan["chunk_wfirst"]
    chunk_span = plan["chunk_span"]
    w_span = plan["w_span"]
    live = plan["live"]
    half = min(D, 512)
    n_half = D // half
    eq = mybir.AluOpType.is_equal

    nc = bacc.Bacc("TRN2", target_bir_lowering=False, debug=False,
                   enable_asserts=False, num_devices=n_cores)
    rows_d = nc.dram_tensor("rows", [P, (npad // P) * D], F16,
                            kind="ExternalInput").ap()
    meta_d = nc.dram_tensor("meta", [P, nchunk * 2], F32,
                            kind="ExternalInput").ap()
    iota_d = nc.dram_tensor("iota", [P, w_span * w_tok], F16,
                            kind="ExternalInput").ap()
    # out[p, w*D + d] = token (w*w_tok + p), feature d; host re-layouts.
    out_d = nc.dram_tensor("out", [P, n_win * D], F16,
                           kind="ExternalOutput").ap()

    with tile.TileContext(nc) as tc:
        with (
            tc.tile_pool(name="grp", bufs=group_bufs) as gpool,
            tc.tile_pool(name="misc", bufs=1) as mpool,
            tc.tile_pool(name="stage", bufs=stage_bufs) as spool,
            tc.tile_pool(name="oh", bufs=onehot_bufs) as opool,
            tc.tile_pool(name="ps", bufs=psum_bufs, space="PSUM") as ppool,
        ):
            iota_t = mpool.tile([P, w_span * w_tok], F16)
            nc.gpsimd.dma_start(out=iota_t[:], in_=iota_d[:])
            meta_t = mpool.tile([P, nchunk * 2], F32)
            nc.gpsimd.dma_start(out=meta_t[:], in_=meta_d[:])

            group_tiles = {}
            oh_tiles = {}

            def get_group(g):
                t = group_tiles.get(g)
                if t is None:
                    t = gpool.tile([P, gch * D], F16, tag="grp")
                    base = g * gch * D
                    # chunks >= live never reach a matmul; skip their DMA.
                    # (their buffer slot holds finite leftovers, harmless)
                    nliv = min(gch, live - g * gch)
                    # inputs ride the sync (SP) HWDGE ring exclusively:
                    # sharing a ring with stores would make later input
                    # loads queue behind compute-dependent semaphores
                    if g < 2:
                        # per-chunk loads so chunk 0 lands ASAP
                        for j in range(nliv):
                            nc.sync.dma_start(
                                out=t[:, j * D:(j + 1) * D],
                                in_=rows_d[:, base + j * D:base + (j + 1) * D],
                            )
                    elif nliv > 0:
                        nc.sync.dma_start(
                            out=t[:, :nliv * D],
                            in_=rows_d[:, base:base + nliv * D],
                        )
                    group_tiles[g] = t
                return t

            def get_oh(c):
                """Weighted one-hot for chunk c over its window span."""
                t = oh_tiles.get(c)
                if t is None:
                    t = opool.tile([P, w_span * w_tok], F16, tag="oh")
                    ncols = chunk_span.get(c, 1) * w_tok
                    nc.vector.tensor_scalar(
                        t[:, :ncols], iota_t[:, :ncols],
                        meta_t[:, nchunk + c:nchunk + c + 1],
                        meta_t[:, c:c + 1],
                        op0=eq, op1=mybir.AluOpType.mult,
                    )
                    oh_tiles[c] = t
                return t

            # store groups: big batched stores, tapered at the end so the
            # final window's store is small (shorter drain tail)
            sizes = _store_sizes(n_win, store_w)

            w = 0
            for sg, sz in enumerate(sizes):
                st = spool.tile([P, store_w * D], F16, tag="st")
                for sw in range(sz):
                    ps = ppool.tile([P, D], F32)
                    s, e = win_pair_slices[w]
                    for j in range(s, e):
                        _, c = pairs[j]
                        first, last = (j == s), (j == e - 1)
                        oh = get_oh(c)
                        g, k = divmod(c, gch)
                        gt = get_group(g)
                        off = (w - chunk_wfirst[c]) * w_tok
                        ohs = oh[:, off:off + w_tok]
                        for h in range(n_half):
                            hs = slice(h * half, (h + 1) * half)
                            nc.tensor.matmul(ps[:, hs],
                                             ohs,
                                             gt[:, k * D + h * half:
                                                k * D + (h + 1) * half],
                                             start=first, stop=last)
                    # fp32 PSUM -> fp16 SBUF; VectorE bank 0, ScalarE bank 1
                    hd = D // 2
                    nc.vector.tensor_copy(st[:, sw * D:sw * D + hd],
                                          ps[:, :hd])
                    nc.scalar.activation(st[:, sw * D + hd:(sw + 1) * D],
                                         ps[:, hd:],
                                         mybir.ActivationFunctionType.Copy)
                    w += 1
                nc.scalar.dma_start(
                    out=out_d[:, (w - sz) * D:w * D], in_=st[:, :sz * D])

    nc.compile()
    return nc


def kernel(expert_outputs, weights, token_indices, batch_size, seq_len):
    expert_outputs = np.ascontiguousarray(expert_outputs, dtype=np.float32)
    weights = np.ascontiguousarray(weights, dtype=np.float32)
    B, S = int(batch_size), int(seq_len)
    E, C, D = expert_outputs.shape
    n_tokens = B * S

    x_flat = expert_outputs.reshape(-1, D)
    w_flat = weights.reshape(-1)
    idx_flat = np.asarray(token_indices).reshape(-1).astype(np.int64)

    plan = _make_plan(idx_flat, n_tokens, N_CORES)
    in_maps = [_pack_core_inputs(plan, m, x_flat, w_flat, D)
               for m in range(N_CORES)]
    nc = _build_program(plan, D, N_CORES)

    res = bass_utils.run_bass_kernel_spmd(
        nc, in_maps, core_ids=list(range(N_CORES)), trace=False,
    )
    tok_per_core = plan["tok_per_core"]
    n_win = plan["n_win"]
    out = np.empty((n_tokens, D), np.float32)
    for m in range(N_CORES):
        o = res.results[m]["out"]  # [P, n_win*D] fp16
        out[m * tok_per_core:(m + 1) * tok_per_core] = (
            np.asarray(o).reshape(P, n_win, D)
            .transpose(1, 0, 2).reshape(tok_per_core, D)
        )
    return out.reshape(B, S, D)
